# revision 1
# baseline (speedup 1.0000x reference)
"""Two-layer GCN (ClinicalGCN) on 8 Trainium2 NeuronCores.

Math (fold the symmetric GCN norm into node features; b1/b2 handled
separately, and when they are zero — as in this problem — fused away):
    h_hat[v]   = (x @ W1)[v] * dinv[v]
    agg1[i]    = sum_{e: dst=i} h_hat[src[e]]         (segment sum)
    h1_hat[v]  = dinv[v] * relu(dinv[v]*agg1[v] + b1) -> bf16 table
    agg2[i]    = sum_{e: dst=i} h1_hat[src[e]]
    out[i]     = (dinv[i]*agg2[i]) @ W2 + b2

Device mapping:
  - dst-shard nodes across 8 cores; per-core 49 blocks of 128 dst nodes.
  - Features tables ([50176,128] bf16) are AllGather'd; source rows are
    fetched with gpsimd.dma_gather (int16 indices -> table split in two
    25088-row halves).
  - Per 128-edge chunk, a 0/1 selection matrix S (built with one DVE
    is_equal per block) routes messages to dst rows via PE matmul
    accumulation in PSUM.
"""

import math

import ml_dtypes
import numpy as np

import concourse.bacc as bacc
import concourse.bass as bass
import concourse.mybir as mybir
import concourse.tile as tile
from concourse.bass_utils import run_bass_kernel_spmd

P = 128
N_CORES = 8
BF16 = ml_dtypes.bfloat16


class Cfg:
    def __init__(self, n_nodes, n_in, n_hid, n_out, n_cores=N_CORES):
        assert n_nodes % n_cores == 0
        self.n = n_nodes
        self.nin = n_in
        self.nh = n_hid
        self.nc_out = n_out
        self.cores = n_cores
        self.shard = n_nodes // n_cores           # real nodes per core
        self.nblk = (self.shard + P - 1) // P     # dst blocks per core
        self.pshard = self.nblk * P               # padded nodes per core
        self.tabn = self.pshard * n_cores         # gather-table rows
        assert self.tabn % 2 == 0 and (self.tabn // 2) % self.pshard == 0
        self.half = self.tabn // 2                # rows per table half
        assert self.half <= 32768, "int16 dma_gather index limit"
        self.kin = n_in // P                      # k chunks for x @ W1


FULL = Cfg(50000, 256, 128, 4)


# ---------------------------------------------------------------- host prep
def host_prep(cfg: Cfg, x, edge_index, W1, b1, W2, b2):
    """Build per-core input arrays. Pure numpy."""
    n = cfg.n
    src = np.concatenate([edge_index[0], np.arange(n, dtype=np.int64)])
    dst = np.concatenate([edge_index[1], np.arange(n, dtype=np.int64)])
    deg = np.bincount(dst, minlength=n).astype(np.float32)
    dinv = np.where(deg > 0, 1.0 / np.sqrt(deg), 0.0).astype(np.float32)

    # table row index for each global node id
    trow = ((src // cfg.shard) * cfg.pshard + src % cfg.shard).astype(np.int64)

    # order edges by destination; dst = core*shard + local so this groups
    # by (core, block) with our local block definition
    order = np.argsort(dst, kind="stable")
    dst_s = dst[order]
    trow_s = trow[order]
    ldl_s = dst_s % cfg.shard
    lslot_s = (ldl_s % P).astype(np.float32)
    half_s = (trow_s >= cfg.half).astype(np.int64)
    blk_s = (dst_s // cfg.shard) * cfg.nblk + ldl_s // P

    nblk_total = cfg.cores * cfg.nblk
    # chunk counts per (block, half); K per LOCAL block = max across cores
    # (the SPMD program is shared, so per-block sizes must agree per core)
    cnt = np.zeros((nblk_total, 2), dtype=np.int64)
    np.add.at(cnt, (blk_s, half_s), 1)
    cnt3 = cnt.reshape(cfg.cores, cfg.nblk, 2)
    KH = [np.maximum(1, np.ceil(cnt3[:, :, h].max(axis=0) / P)).astype(int)
          for h in range(2)]  # each: [nblk]

    # bucket sort edges by (block, half)
    key = blk_s * 2 + half_s
    order2 = np.argsort(key, kind="stable")
    trow2 = trow_s[order2]
    lslot2 = lslot_s[order2]
    key2 = key[order2]
    starts = np.searchsorted(key2, np.arange(nblk_total * 2 + 1))

    # ragged flat layouts with host-known offsets
    goff = [np.concatenate([[0], np.cumsum(KH[h] * P * 8)]) for h in range(2)]
    Ksum = KH[0] + KH[1]
    loff = np.concatenate([[0], np.cumsum(Ksum * P)])

    per_core = []
    for c in range(cfg.cores):
        gidx = [np.zeros(goff[h][-1], dtype=np.int16) for h in range(2)]
        ldst = np.full(loff[-1], -1.0, dtype=BF16)
        for b in range(cfg.nblk):
            g = c * cfg.nblk + b
            ld_b = np.full((P, Ksum[b]), -1.0, dtype=BF16)
            for h in range(2):
                lo, hi = starts[g * 2 + h], starts[g * 2 + h + 1]
                cnt_e = hi - lo
                tr = trow2[lo:hi] - h * cfg.half
                ls = lslot2[lo:hi]
                idx = np.zeros(KH[h][b] * P, dtype=np.int16)
                idx[:cnt_e] = tr
                wrapped = idx.reshape(KH[h][b] * 8, 16).T   # [16, K*8]
                gidx[h][goff[h][b]:goff[h][b + 1]] = \
                    np.tile(wrapped, (8, 1)).ravel()        # replicate
                t = np.arange(cnt_e)
                j0 = 0 if h == 0 else KH[0][b]
                ld_b[t % P, j0 + t // P] = ls.astype(BF16)
            ldst[loff[b]:loff[b + 1]] = ld_b.ravel()
        xs = x[c * cfg.shard:(c + 1) * cfg.shard]
        xT = np.zeros((cfg.nin, cfg.pshard), dtype=BF16)
        xT[:, :cfg.shard] = xs.T.astype(BF16)
        dv = np.zeros((cfg.pshard, 1), dtype=np.float32)
        dv[:cfg.shard, 0] = dinv[c * cfg.shard:(c + 1) * cfg.shard]
        per_core.append({
            "xT": xT,
            "dinv": dv,
            "dinv2": dv * dv,
            "gidxA": gidx[0],
            "gidxB": gidx[1],
            "ldst": ldst,
        })

    iota = np.broadcast_to(np.arange(P, dtype=np.float32).astype(BF16),
                           (P, P)).copy()
    ident = np.eye(P, dtype=np.float32).astype(BF16)
    shared = {
        "W1": W1.astype(BF16),
        "W2": W2.astype(BF16),
        "b1r": np.broadcast_to(b1.astype(np.float32), (P, cfg.nh)).copy(),
        "b2r": np.broadcast_to(b2.astype(np.float32), (P, cfg.nc_out)).copy(),
        "iota": iota,
        "ident": ident,
    }
    in_maps = [{**shared, **pc} for pc in per_core]
    zero_bias = not (np.any(b1) or np.any(b2))
    return in_maps, KH, zero_bias


# --------------------------------------------------------------- bass build
def build_nc(cfg: Cfg, KH, zero_bias):
    f32 = mybir.dt.float32
    bf16 = mybir.dt.bfloat16
    i16 = mybir.dt.int16
    KA, KB = KH                      # per-block chunk counts, [nblk] each
    Ksum = [int(KA[b] + KB[b]) for b in range(cfg.nblk)]
    goffA = np.concatenate([[0], np.cumsum(np.asarray(KA) * P * 8)])
    goffB = np.concatenate([[0], np.cumsum(np.asarray(KB) * P * 8)])
    loff = np.concatenate([[0], np.cumsum(np.asarray(Ksum) * P)])

    nc = bacc.Bacc("TRN2", target_bir_lowering=False, debug=False,
                   num_devices=cfg.cores)

    xT = nc.dram_tensor("xT", [cfg.nin, cfg.pshard], bf16,
                        kind="ExternalInput")
    W1 = nc.dram_tensor("W1", [cfg.nin, cfg.nh], bf16, kind="ExternalInput")
    W2 = nc.dram_tensor("W2", [cfg.nh, cfg.nc_out], bf16, kind="ExternalInput")
    b1r = nc.dram_tensor("b1r", [P, cfg.nh], f32, kind="ExternalInput")
    b2r = nc.dram_tensor("b2r", [P, cfg.nc_out], f32, kind="ExternalInput")
    dinv = nc.dram_tensor("dinv", [cfg.pshard, 1], f32, kind="ExternalInput")
    dinv2 = nc.dram_tensor("dinv2", [cfg.pshard, 1], f32, kind="ExternalInput")
    iota = nc.dram_tensor("iota", [P, P], bf16, kind="ExternalInput")
    ident = nc.dram_tensor("ident", [P, P], bf16, kind="ExternalInput")
    gidxA = nc.dram_tensor("gidxA", [int(goffA[-1])], i16,
                           kind="ExternalInput")
    gidxB = nc.dram_tensor("gidxB", [int(goffB[-1])], i16,
                           kind="ExternalInput")
    ldst = nc.dram_tensor("ldst", [int(loff[-1])], bf16,
                          kind="ExternalInput")
    out = nc.dram_tensor("out", [cfg.pshard, cfg.nc_out], f32,
                         kind="ExternalOutput")

    with tile.TileContext(nc) as tc:
        with (
            tc.tile_pool(name="const", bufs=1) as cpool,
            tc.tile_pool(name="x", bufs=3) as xpool,
            tc.tile_pool(name="h", bufs=3) as hpool,
            tc.tile_pool(name="msg", bufs=3) as mpool,
            tc.tile_pool(name="sel", bufs=3) as spool,
            tc.tile_pool(name="small", bufs=4) as smpool,
            tc.tile_pool(name="ps", bufs=2, space="PSUM") as pspool,
            tc.tile_pool(name="ps2", bufs=1, space="PSUM") as ps2pool,
            tc.tile_pool(name="dram", bufs=1, space="DRAM") as dram,
        ):
            # ---- constants in SBUF (W1 as kin slices of [128, nh])
            w1t = cpool.tile([P, cfg.kin * cfg.nh], bf16, tag="w1")
            nc.sync.dma_start(
                out=w1t[:].rearrange("p (a d) -> p a d", a=cfg.kin),
                in_=W1[:].rearrange("(a p) d -> p a d", p=P))
            # whole xT resident in SBUF: [128, kin, pshard] bf16
            xall = cpool.tile([P, cfg.kin * cfg.pshard], bf16, tag="xall")
            nc.sync.dma_start(
                out=xall[:].rearrange("p (a d) -> p a d", a=cfg.kin),
                in_=xT[:].rearrange("(a p) d -> p a d", p=P))
            w2t = cpool.tile([cfg.nh, cfg.nc_out], bf16, tag="w2")
            nc.sync.dma_start(out=w2t[:], in_=W2[:])
            b1t = cpool.tile([P, cfg.nh], f32, tag="b1")
            nc.sync.dma_start(out=b1t[:], in_=b1r[:])
            b2t = cpool.tile([P, cfg.nc_out], f32, tag="b2")
            nc.sync.dma_start(out=b2t[:], in_=b2r[:])
            iot = cpool.tile([P, P], bf16, tag="iota")
            nc.sync.dma_start(out=iot[:], in_=iota[:])
            idt = cpool.tile([P, P], bf16, tag="ident")
            nc.sync.dma_start(out=idt[:], in_=ident[:])
            dvt = cpool.tile([P, cfg.nblk], f32, tag="dinv")
            nc.sync.dma_start(
                out=dvt[:], in_=dinv[:].rearrange("(j p) one -> p (j one)", p=P))
            dv2t = cpool.tile([P, cfg.nblk], f32, tag="dinv2")
            nc.sync.dma_start(
                out=dv2t[:], in_=dinv2[:].rearrange("(j p) one -> p (j one)", p=P))

            hsh = dram.tile([cfg.pshard, cfg.nh], bf16)
            htab = dram.tile([cfg.tabn, cfg.nh], bf16, addr_space="Shared")
            h1sh = dram.tile([cfg.pshard, cfg.nh], bf16)
            h1tab = dram.tile([cfg.tabn, cfg.nh], bf16, addr_space="Shared")

            # ---------------- phase 1: h_hat = (x @ W1) * dinv -> AllGather
            for t in range(cfg.nblk):
                ps = pspool.tile([P, cfg.nh], f32, tag="ps_h")
                for kk in range(cfg.kin):
                    nc.tensor.matmul(
                        out=ps[:],
                        lhsT=xall[:, kk * cfg.pshard + t * P:
                                  kk * cfg.pshard + (t + 1) * P],
                        rhs=w1t[:, kk * cfg.nh:(kk + 1) * cfg.nh],
                        start=(kk == 0), stop=(kk == cfg.kin - 1))
                hh = hpool.tile([P, cfg.nh], bf16, tag="hh")
                nc.vector.tensor_scalar_mul(out=hh[:], in0=ps[:],
                                            scalar1=dvt[:, t:t + 1])
                nc.sync.dma_start(out=hsh[t * P:(t + 1) * P, :], in_=hh[:])

            nc.gpsimd.collective_compute(
                "AllGather", mybir.AluOpType.bypass,
                replica_groups=[list(range(cfg.cores))],
                ins=[hsh.opt()], outs=[htab.opt()])

            # helper: gather + segment-sum for one block -> psum [P, nh] f32
            Kmax = max(Ksum)

            def gather_agg(b, table, msg_tag, sel_tag, gi_tag):
                K_b = Ksum[b]
                msg = mpool.tile([P, Kmax * cfg.nh], bf16, tag=msg_tag)
                for h, (KHh, gsrc, goff) in enumerate(
                        ((int(KA[b]), gidxA, goffA),
                         (int(KB[b]), gidxB, goffB))):
                    gi = smpool.tile([P, KHh * 8], i16, tag=f"{gi_tag}{h}")
                    nc.sync.dma_start(
                        out=gi[:],
                        in_=gsrc[int(goff[b]):int(goff[b + 1])].rearrange(
                            "(p k) -> p k", p=P))
                    j0 = 0 if h == 0 else int(KA[b])
                    nc.gpsimd.dma_gather(
                        out_ap=msg[:, j0 * cfg.nh:(j0 + KHh) * cfg.nh]
                        .rearrange("p (k f) -> p k f", k=KHh),
                        in_ap=table[h * cfg.half:(h + 1) * cfg.half, :],
                        idxs_ap=gi[:],
                        num_idxs=KHh * P,
                        num_idxs_reg=KHh * P,
                        elem_size=cfg.nh,
                        single_packet=False)
                ldt = smpool.tile([P, K_b], bf16, tag=f"{gi_tag}ld")
                nc.sync.dma_start(
                    out=ldt[:],
                    in_=ldst[int(loff[b]):int(loff[b + 1])].rearrange(
                        "(p k) -> p k", p=P))
                sel = spool.tile([P, Kmax * P], bf16, tag=sel_tag)
                nc.vector.tensor_tensor(
                    out=sel[:, :K_b * P].rearrange("p (k f) -> p k f", k=K_b),
                    in0=ldt[:, :, None].to_broadcast([P, K_b, P]),
                    in1=iot[:, None, :].to_broadcast([P, K_b, P]),
                    op=mybir.AluOpType.is_equal)
                ps = pspool.tile([P, cfg.nh], f32, tag="ps_agg")
                for j in range(K_b):
                    nc.tensor.matmul(
                        out=ps[:], lhsT=sel[:, j * P:(j + 1) * P],
                        rhs=msg[:, j * cfg.nh:(j + 1) * cfg.nh],
                        start=(j == 0), stop=(j == K_b - 1))
                return ps

            # ---------------- phase 2: h1_hat table
            for b in range(cfg.nblk):
                ps = gather_agg(b, htab, "msg2", "sel2", "gi2")
                hh = hpool.tile([P, cfg.nh], bf16, tag="h1h")
                if zero_bias:
                    # h1_hat = dinv^2 * relu(agg)   (dinv>0, b1=0)
                    nc.vector.tensor_scalar(
                        out=hh[:], in0=ps[:], scalar1=0.0,
                        scalar2=dv2t[:, b:b + 1],
                        op0=mybir.AluOpType.max, op1=mybir.AluOpType.mult)
                else:
                    t1 = hpool.tile([P, cfg.nh], f32, tag="h1f")
                    nc.vector.tensor_scalar_mul(out=t1[:], in0=ps[:],
                                                scalar1=dvt[:, b:b + 1])
                    nc.vector.tensor_add(out=t1[:], in0=t1[:], in1=b1t[:])
                    nc.vector.tensor_scalar(
                        out=hh[:], in0=t1[:], scalar1=0.0,
                        scalar2=dvt[:, b:b + 1],
                        op0=mybir.AluOpType.max, op1=mybir.AluOpType.mult)
                nc.sync.dma_start(out=h1sh[b * P:(b + 1) * P, :], in_=hh[:])

            nc.gpsimd.collective_compute(
                "AllGather", mybir.AluOpType.bypass,
                replica_groups=[list(range(cfg.cores))],
                ins=[h1sh.opt()], outs=[h1tab.opt()])

            # ---------------- phase 3: out = (dinv*agg2) @ W2 (+ b2)
            for b in range(cfg.nblk):
                ps = gather_agg(b, h1tab, "msg3", "sel3", "gi3")
                c1 = hpool.tile([P, cfg.nh], bf16, tag="c1")
                nc.vector.tensor_scalar_mul(out=c1[:], in0=ps[:],
                                            scalar1=dvt[:, b:b + 1])
                pst = ps2pool.tile([P, cfg.nh], bf16, tag="ps_t")
                nc.tensor.transpose(out=pst[:], in_=c1[:], identity=idt[:])
                aggT = hpool.tile([P, cfg.nh], bf16, tag="aggT")
                nc.vector.tensor_copy(out=aggT[:], in_=pst[:])
                pso = ps2pool.tile([P, cfg.nc_out], f32, tag="ps_o")
                nc.tensor.matmul(out=pso[:], lhsT=aggT[:], rhs=w2t[:],
                                 start=True, stop=True)
                ot = hpool.tile([P, cfg.nc_out], f32, tag="ot")
                if zero_bias:
                    nc.vector.tensor_copy(out=ot[:], in_=pso[:])
                else:
                    nc.vector.tensor_add(out=ot[:], in0=pso[:], in1=b2t[:])
                nc.sync.dma_start(out=out[b * P:(b + 1) * P, :], in_=ot[:])

    nc.compile()
    return nc


# ------------------------------------------------------------------ driver
def kernel(x, edge_index, W1, b1, W2, b2):
    cfg = FULL
    assert x.shape == (cfg.n, cfg.nin)
    in_maps, KH, zero_bias = host_prep(
        cfg, np.asarray(x), np.asarray(edge_index), np.asarray(W1),
        np.asarray(b1), np.asarray(W2), np.asarray(b2))
    nc = build_nc(cfg, KH, zero_bias)
    res = run_bass_kernel_spmd(nc, in_maps, core_ids=list(range(cfg.cores)))
    parts = [res.results[c]["out"][:cfg.shard] for c in range(cfg.cores)]
    return np.concatenate(parts, axis=0).astype(np.float32)



# revision 18
# speedup vs baseline: 2.7244x; 2.7244x over previous
"""Two-layer GCN (ClinicalGCN) on 8 Trainium2 NeuronCores.

Math (fold the symmetric GCN norm into node features; self-loops handled
algebraically, not gathered; b1/b2 handled separately, and when they are
zero — as in this problem — fused away):
    h_hat[v]   = (x @ W1)[v] * dinv[v]
    agg1'[i]   = sum_{real e: dst=i} h_hat[src[e]]      (segment sum)
    h1_hat[v]  = dinv[v] * relu(dinv[v]*(agg1'[v] + h_hat[v]) + b1)
    agg2'[i]   = sum_{real e: dst=i} h1_hat[src[e]]
    out[i]     = (dinv[i]*(agg2'[i] + h1_hat[i])) @ W2 + b2

Device mapping:
  - dst-shard nodes across 8 cores; per-core 49 blocks of 128 dst nodes.
  - Feature tables ([50176,128] bf16) are AllGather'd; source rows are
    fetched with gpsimd.dma_gather (int16 indices -> table split in two
    25088-row halves).  Gather calls round-robin across 4 SWDGE queues so
    descriptor generation runs on all 8 Q7 cores (queue q uses core pair
    2q/2q+1) instead of serializing on cores 0-1.
  - Trailing padding indices are -1: the gather ucode truncates trailing
    negatives, so padding costs no descriptor-generation or DMA time.
    Padding slots have sel==0 so stale msg data is multiplied by zero
    (msg buffers are memset once at startup so stale data is never NaN).
  - Per 128-edge chunk, a 0/1 selection matrix S (built with one DVE
    is_equal per block) routes messages to dst rows via PE matmul
    accumulation in PSUM.
"""

import math

import ml_dtypes
import numpy as np

import concourse.bacc as bacc
import concourse.bass as bass
import concourse.mybir as mybir
import concourse.tile as tile
from concourse.bass_utils import run_bass_kernel_spmd

P = 128
N_CORES = 8
N_QUEUES = 4
BF16 = ml_dtypes.bfloat16


class Cfg:
    def __init__(self, n_nodes, n_in, n_hid, n_out, n_cores=N_CORES):
        assert n_nodes % n_cores == 0
        self.n = n_nodes
        self.nin = n_in
        self.nh = n_hid
        self.nc_out = n_out
        self.cores = n_cores
        self.shard = n_nodes // n_cores           # real nodes per core
        self.nblk = (self.shard + P - 1) // P     # dst blocks per core
        self.pshard = self.nblk * P               # padded nodes per core
        self.tabn = self.pshard * n_cores         # gather-table rows
        assert self.tabn % 2 == 0 and (self.tabn // 2) % self.pshard == 0
        self.half = self.tabn // 2                # rows per table half
        assert self.half <= 32768, "int16 dma_gather index limit"
        self.kin = n_in // P                      # k chunks for x @ W1


FULL = Cfg(50000, 256, 128, 4)


# ---------------------------------------------------------------- host prep
def host_prep(cfg: Cfg, x, edge_index, W1, b1, W2, b2):
    """Build per-core input arrays. Pure numpy."""
    n = cfg.n
    # degrees/norm include the self loop (GCN: deg = indeg + 1)
    dst_all = np.concatenate([edge_index[1], np.arange(n, dtype=np.int64)])
    deg = np.bincount(dst_all, minlength=n).astype(np.float32)
    dinv = np.where(deg > 0, 1.0 / np.sqrt(deg), 0.0).astype(np.float32)

    # only real edges are gathered; self loops are added algebraically
    src = edge_index[0].astype(np.int64)
    dst = edge_index[1].astype(np.int64)

    # table row index for each global node id
    trow = ((src // cfg.shard) * cfg.pshard + src % cfg.shard).astype(np.int64)

    # order edges by destination; dst = core*shard + local so this groups
    # by (core, block) with our local block definition
    order = np.argsort(dst, kind="stable")
    dst_s = dst[order]
    trow_s = trow[order]
    ldl_s = dst_s % cfg.shard
    lslot_s = (ldl_s % P).astype(np.float32)
    half_s = (trow_s >= cfg.half).astype(np.int64)
    blk_s = (dst_s // cfg.shard) * cfg.nblk + ldl_s // P

    nblk_total = cfg.cores * cfg.nblk
    # chunk counts per (block, half); K per LOCAL block = max across cores
    # (the SPMD program is shared, so per-block sizes must agree per core)
    cnt = np.zeros((nblk_total, 2), dtype=np.int64)
    np.add.at(cnt, (blk_s, half_s), 1)
    cnt3 = cnt.reshape(cfg.cores, cfg.nblk, 2)
    # shared valid-index count per (block, half): max across cores (the
    # SPMD program passes this as num_idxs_reg, so it must agree per core)
    Vmax = np.maximum(1, cnt3.max(axis=0))            # [nblk, 2]
    KH = [np.maximum(1, np.ceil(Vmax[:, h] / P)).astype(int)
          for h in range(2)]  # each: [nblk]

    # bucket sort edges by (block, half)
    key = blk_s * 2 + half_s
    order2 = np.argsort(key, kind="stable")
    trow2 = trow_s[order2]
    lslot2 = lslot_s[order2]
    key2 = key[order2]
    starts = np.searchsorted(key2, np.arange(nblk_total * 2 + 1))

    # ragged flat layouts with host-known offsets
    goff = [np.concatenate([[0], np.cumsum(KH[h] * P * 8)]) for h in range(2)]
    Ksum = KH[0] + KH[1]
    loff = np.concatenate([[0], np.cumsum(Ksum * P)])

    per_core = []
    for c in range(cfg.cores):
        gidx = [np.zeros(goff[h][-1], dtype=np.int16) for h in range(2)]
        ldst = np.full(loff[-1], -1.0, dtype=BF16)
        for b in range(cfg.nblk):
            g = c * cfg.nblk + b
            ld_b = np.full((P, Ksum[b]), -1.0, dtype=BF16)
            for h in range(2):
                lo, hi = starts[g * 2 + h], starts[g * 2 + h + 1]
                cnt_e = hi - lo
                tr = trow2[lo:hi] - h * cfg.half
                ls = lslot2[lo:hi]
                # [real | dummy 0s up to shared Vmax | -1 padding]: the
                # gather ucode truncates trailing negatives, so slots past
                # Vmax cost no descgen/DMA time; dummy 0s keep the valid
                # count identical across cores (num_idxs_reg is shared)
                idx = np.full(KH[h][b] * P, -1, dtype=np.int16)
                idx[:cnt_e] = tr
                idx[cnt_e:Vmax[b, h]] = 0
                wrapped = idx.reshape(KH[h][b] * 8, 16).T   # [16, K*8]
                gidx[h][goff[h][b]:goff[h][b + 1]] = \
                    np.tile(wrapped, (8, 1)).ravel()        # replicate
                t = np.arange(cnt_e)
                j0 = 0 if h == 0 else KH[0][b]
                ld_b[t % P, j0 + t // P] = ls.astype(BF16)
            ldst[loff[b]:loff[b + 1]] = ld_b.ravel()
        xs = x[c * cfg.shard:(c + 1) * cfg.shard]
        xT = np.zeros((cfg.nin, cfg.pshard), dtype=BF16)
        xT[:, :cfg.shard] = xs.T.astype(BF16)
        dv = np.zeros((cfg.pshard, 1), dtype=np.float32)
        dv[:cfg.shard, 0] = dinv[c * cfg.shard:(c + 1) * cfg.shard]
        per_core.append({
            "xT": xT,
            "dinv": dv,
            "dinv2": dv * dv,
            "gidxA": gidx[0],
            "gidxB": gidx[1],
            "ldst": ldst,
        })

    Kmax = int(Ksum.max())
    iota = np.broadcast_to(np.arange(P, dtype=np.float32).astype(BF16),
                           (P, P))
    iota_big = np.tile(iota, (1, Kmax)).copy()   # [P, Kmax*P]
    ident = np.eye(P, dtype=np.float32).astype(BF16)
    shared = {
        "W1": W1.astype(BF16),
        "W2": W2.astype(BF16),
        "b1r": np.broadcast_to(b1.astype(np.float32), (P, cfg.nh)).copy(),
        "b2r": np.broadcast_to(b2.astype(np.float32), (P, cfg.nc_out)).copy(),
        "iotab": iota_big,
        "ident": ident,
    }
    in_maps = [{**shared, **pc} for pc in per_core]
    zero_bias = not (np.any(b1) or np.any(b2))
    return in_maps, (KH[0], KH[1], Vmax), zero_bias


# --------------------------------------------------------------- bass build
def build_nc(cfg: Cfg, KH, zero_bias):
    f32 = mybir.dt.float32
    bf16 = mybir.dt.bfloat16
    i16 = mybir.dt.int16
    KA, KB, Vmax = KH                # per-block chunk counts, [nblk] each
    Ksum = [int(KA[b] + KB[b]) for b in range(cfg.nblk)]
    goffA = np.concatenate([[0], np.cumsum(np.asarray(KA) * P * 8)])
    goffB = np.concatenate([[0], np.cumsum(np.asarray(KB) * P * 8)])
    loff = np.concatenate([[0], np.cumsum(np.asarray(Ksum) * P)])
    Kmax = max(Ksum)

    nc = bacc.Bacc("TRN2", target_bir_lowering=False, debug=False,
                   num_devices=cfg.cores, num_swdge_queues=N_QUEUES)

    xT = nc.dram_tensor("xT", [cfg.nin, cfg.pshard], bf16,
                        kind="ExternalInput")
    W1 = nc.dram_tensor("W1", [cfg.nin, cfg.nh], bf16, kind="ExternalInput")
    W2 = nc.dram_tensor("W2", [cfg.nh, cfg.nc_out], bf16, kind="ExternalInput")
    b1r = nc.dram_tensor("b1r", [P, cfg.nh], f32, kind="ExternalInput")
    b2r = nc.dram_tensor("b2r", [P, cfg.nc_out], f32, kind="ExternalInput")
    dinv = nc.dram_tensor("dinv", [cfg.pshard, 1], f32, kind="ExternalInput")
    dinv2 = nc.dram_tensor("dinv2", [cfg.pshard, 1], f32, kind="ExternalInput")
    iotab = nc.dram_tensor("iotab", [P, Kmax * P], bf16, kind="ExternalInput")
    ident = nc.dram_tensor("ident", [P, P], bf16, kind="ExternalInput")
    gidxA = nc.dram_tensor("gidxA", [int(goffA[-1])], i16,
                           kind="ExternalInput")
    gidxB = nc.dram_tensor("gidxB", [int(goffB[-1])], i16,
                           kind="ExternalInput")
    ldst = nc.dram_tensor("ldst", [int(loff[-1])], bf16,
                          kind="ExternalInput")
    out = nc.dram_tensor("out", [cfg.pshard, cfg.nc_out], f32,
                         kind="ExternalOutput")

    qctr = [0]

    def next_q():
        q = qctr[0] % N_QUEUES
        qctr[0] += 1
        return q

    with tile.TileContext(nc) as tc:
        with (
            tc.tile_pool(name="const", bufs=1) as cpool,
            tc.tile_pool(name="h", bufs=3) as hpool,
            tc.tile_pool(name="sel", bufs=4) as spool,
            tc.tile_pool(name="small", bufs=8) as smpool,
            tc.tile_pool(name="ps", bufs=4, space="PSUM") as pspool,
            tc.tile_pool(name="ps2", bufs=2, space="PSUM") as ps2pool,
            tc.tile_pool(name="dram", bufs=1, space="DRAM") as dram,
        ):
            # ---- constants in SBUF (W1 as kin slices of [128, nh])
            w1t = cpool.tile([P, cfg.kin * cfg.nh], bf16, tag="w1")
            nc.sync.dma_start(
                out=w1t[:].rearrange("p (a d) -> p a d", a=cfg.kin),
                in_=W1[:].rearrange("(a p) d -> p a d", p=P))
            # whole xT resident in SBUF: [128, kin, pshard] bf16
            xall = cpool.tile([P, cfg.kin * cfg.pshard], bf16, tag="xall")
            nc.sync.dma_start(
                out=xall[:].rearrange("p (a d) -> p a d", a=cfg.kin),
                in_=xT[:].rearrange("(a p) d -> p a d", p=P))
            w2t = cpool.tile([cfg.nh, cfg.nc_out], bf16, tag="w2")
            nc.sync.dma_start(out=w2t[:], in_=W2[:])
            b1t = cpool.tile([P, cfg.nh], f32, tag="b1")
            nc.sync.dma_start(out=b1t[:], in_=b1r[:])
            b2t = cpool.tile([P, cfg.nc_out], f32, tag="b2")
            nc.sync.dma_start(out=b2t[:], in_=b2r[:])
            iot = cpool.tile([P, Kmax * P], bf16, tag="iotab")
            nc.sync.dma_start(out=iot[:], in_=iotab[:])
            idt = cpool.tile([P, P], bf16, tag="ident")
            nc.sync.dma_start(out=idt[:], in_=ident[:])
            dvt = cpool.tile([P, cfg.nblk], f32, tag="dinv")
            nc.sync.dma_start(
                out=dvt[:], in_=dinv[:].rearrange("(j p) one -> p (j one)", p=P))
            dv2t = cpool.tile([P, cfg.nblk], f32, tag="dinv2")
            nc.sync.dma_start(
                out=dv2t[:], in_=dinv2[:].rearrange("(j p) one -> p (j one)", p=P))

            # resident h_hat / h1_hat blocks (self-loop terms)
            hhall = cpool.tile([P, cfg.nblk * cfg.nh], bf16, tag="hhall")
            h1all = cpool.tile([P, cfg.nblk * cfg.nh], bf16, tag="h1all")

            hsh = dram.tile([cfg.pshard, cfg.nh], bf16)
            htab = dram.tile([cfg.tabn, cfg.nh], bf16, addr_space="Shared")
            h1sh = dram.tile([cfg.pshard, cfg.nh], bf16)
            h1tab = dram.tile([cfg.tabn, cfg.nh], bf16, addr_space="Shared")

            # Persistent msg buffers, zero-filled once: with -1 index padding
            # the gather skips padding slots, so stale buffer contents must be
            # finite (sel==0 kills them in the matmul, but 0*NaN would be NaN).
            # Persistent tiles (not pool-rotated) make every access hit the
            # same logical tile, so Tile emits the write-after-read sems.
            NMSG = 6
            msgbufs = []
            for i in range(NMSG):
                mz = cpool.tile([P, Kmax * cfg.nh], bf16, tag=f"msgb{i}")
                nc.vector.memset(mz[:], 0.0)
                msgbufs.append(mz)
            mctr = [0]

            # ---------------- phase 1: h_hat = (x @ W1) * dinv -> AllGather
            for t in range(cfg.nblk):
                ps = pspool.tile([P, cfg.nh], f32, tag="ps_agg")
                for kk in range(cfg.kin):
                    nc.tensor.matmul(
                        out=ps[:],
                        lhsT=xall[:, kk * cfg.pshard + t * P:
                                  kk * cfg.pshard + (t + 1) * P],
                        rhs=w1t[:, kk * cfg.nh:(kk + 1) * cfg.nh],
                        start=(kk == 0), stop=(kk == cfg.kin - 1))
                nc.vector.tensor_scalar_mul(
                    out=hhall[:, t * cfg.nh:(t + 1) * cfg.nh],
                    in0=ps[:], scalar1=dvt[:, t:t + 1])
                nc.sync.dma_start(out=hsh[t * P:(t + 1) * P, :],
                                  in_=hhall[:, t * cfg.nh:(t + 1) * cfg.nh])

            nc.gpsimd.collective_compute(
                "AllGather", mybir.AluOpType.bypass,
                replica_groups=[list(range(cfg.cores))],
                ins=[hsh.opt()], outs=[htab.opt()])

            # helper: gather + segment-sum for one block -> psum [P, nh] f32
            def gather_agg(b, table, gi_tag):
                K_b = Ksum[b]
                msg = msgbufs[mctr[0] % NMSG]
                mctr[0] += 1
                for h, (KHh, gsrc, goff) in enumerate(
                        ((int(KA[b]), gidxA, goffA),
                         (int(KB[b]), gidxB, goffB))):
                    gi = smpool.tile([P, KHh * 8], i16, tag=f"{gi_tag}{h}")
                    nc.sync.dma_start(
                        out=gi[:],
                        in_=gsrc[int(goff[b]):int(goff[b + 1])].rearrange(
                            "(p k) -> p k", p=P))
                    j0 = 0 if h == 0 else int(KA[b])
                    nc.gpsimd.dma_gather(
                        out_ap=msg[:, j0 * cfg.nh:(j0 + KHh) * cfg.nh]
                        .rearrange("p (k f) -> p k f", k=KHh),
                        in_ap=table[h * cfg.half:(h + 1) * cfg.half, :],
                        idxs_ap=gi[:],
                        num_idxs=KHh * P,
                        num_idxs_reg=int(Vmax[b, h]),
                        elem_size=cfg.nh,
                        single_packet=False,
                        queue_num=next_q())
                ldt = smpool.tile([P, K_b], bf16, tag=f"{gi_tag}ld")
                nc.sync.dma_start(
                    out=ldt[:],
                    in_=ldst[int(loff[b]):int(loff[b + 1])].rearrange(
                        "(p k) -> p k", p=P))
                sel = spool.tile([P, Kmax * P], bf16, tag="sel")
                nc.vector.tensor_tensor(
                    out=sel[:, :K_b * P].rearrange("p (k f) -> p k f", k=K_b),
                    in0=ldt[:, :, None].to_broadcast([P, K_b, P]),
                    in1=iot[:, :K_b * P].rearrange("p (k f) -> p k f", k=K_b),
                    op=mybir.AluOpType.is_equal)
                ps = pspool.tile([P, cfg.nh], f32, tag="ps_agg")
                for j in range(K_b):
                    nc.tensor.matmul(
                        out=ps[:], lhsT=sel[:, j * P:(j + 1) * P],
                        rhs=msg[:, j * cfg.nh:(j + 1) * cfg.nh],
                        start=(j == 0), stop=(j == K_b - 1))
                return ps

            # ---------------- phase 2: h1_hat table
            for b in range(cfg.nblk):
                ps = gather_agg(b, htab, "gi2")
                # full agg = edge agg + self-loop term h_hat[b]
                t0 = hpool.tile([P, cfg.nh], f32, tag="h1f")
                nc.vector.tensor_add(
                    out=t0[:], in0=ps[:],
                    in1=hhall[:, b * cfg.nh:(b + 1) * cfg.nh])
                h1slice = h1all[:, b * cfg.nh:(b + 1) * cfg.nh]
                if zero_bias:
                    # h1_hat = dinv^2 * relu(agg)   (dinv>0, b1=0)
                    nc.vector.tensor_scalar(
                        out=h1slice, in0=t0[:], scalar1=0.0,
                        scalar2=dv2t[:, b:b + 1],
                        op0=mybir.AluOpType.max, op1=mybir.AluOpType.mult)
                else:
                    nc.vector.tensor_scalar_mul(out=t0[:], in0=t0[:],
                                                scalar1=dvt[:, b:b + 1])
                    nc.vector.tensor_add(out=t0[:], in0=t0[:], in1=b1t[:])
                    nc.vector.tensor_scalar(
                        out=h1slice, in0=t0[:], scalar1=0.0,
                        scalar2=dvt[:, b:b + 1],
                        op0=mybir.AluOpType.max, op1=mybir.AluOpType.mult)
                nc.sync.dma_start(out=h1sh[b * P:(b + 1) * P, :], in_=h1slice)

            nc.gpsimd.collective_compute(
                "AllGather", mybir.AluOpType.bypass,
                replica_groups=[list(range(cfg.cores))],
                ins=[h1sh.opt()], outs=[h1tab.opt()])

            # ---------------- phase 3: out = (dinv*(agg2'+h1_hat)) @ W2 (+ b2)
            for b in range(cfg.nblk):
                ps = gather_agg(b, h1tab, "gi3")
                t0 = hpool.tile([P, cfg.nh], f32, tag="c1f")
                nc.vector.tensor_add(
                    out=t0[:], in0=ps[:],
                    in1=h1all[:, b * cfg.nh:(b + 1) * cfg.nh])
                c1 = hpool.tile([P, cfg.nh], bf16, tag="c1")
                nc.vector.tensor_scalar_mul(out=c1[:], in0=t0[:],
                                            scalar1=dvt[:, b:b + 1])
                pst = ps2pool.tile([P, cfg.nh], bf16, tag="ps_t")
                nc.tensor.transpose(out=pst[:], in_=c1[:], identity=idt[:])
                aggT = hpool.tile([P, cfg.nh], bf16, tag="aggT")
                nc.vector.tensor_copy(out=aggT[:], in_=pst[:])
                pso = ps2pool.tile([P, cfg.nc_out], f32, tag="ps_o")
                nc.tensor.matmul(out=pso[:], lhsT=aggT[:], rhs=w2t[:],
                                 start=True, stop=True)
                ot = hpool.tile([P, cfg.nc_out], f32, tag="ot")
                if zero_bias:
                    nc.vector.tensor_copy(out=ot[:], in_=pso[:])
                else:
                    nc.vector.tensor_add(out=ot[:], in0=pso[:], in1=b2t[:])
                nc.sync.dma_start(out=out[b * P:(b + 1) * P, :], in_=ot[:])

    nc.compile()
    return nc


# ------------------------------------------------------------------ driver
def kernel(x, edge_index, W1, b1, W2, b2):
    cfg = FULL
    assert x.shape == (cfg.n, cfg.nin)
    in_maps, KH, zero_bias = host_prep(
        cfg, np.asarray(x), np.asarray(edge_index), np.asarray(W1),
        np.asarray(b1), np.asarray(W2), np.asarray(b2))
    nc = build_nc(cfg, KH, zero_bias)
    res = run_bass_kernel_spmd(nc, in_maps, core_ids=list(range(cfg.cores)))
    parts = [res.results[c]["out"][:cfg.shard] for c in range(cfg.cores)]
    return np.concatenate(parts, axis=0).astype(np.float32)
